# revision 3
# baseline (speedup 1.0000x reference)
"""Trainium2 Bass kernel for nn_Correction_Module_dense.

Math (equivalent to the jax reference):
    g    = x - roll(x, 1, axis=1)            # circular diff along neuron axis
    mask = |g - mean_grad| <= k*sqrt(var_grad)
    y    = x * mask

Sharding: pure data parallel over batch; 8 cores x [512, 8192] slabs.

Per-core pipeline, [128, 1024] chunks (32 chunk-steps).  GPSIMD's walrus
codegen only accepts add/subtract/mult tensor_tensor, so:
    SP   : all DMAs (quarter-granularity x loads; tile-0's first quarter is
           split so compute starts ~5 us in).  xt column 0 holds x[:, N-1]
           (wrap), making g a uniform shifted-AP subtract.
    PE   : per-neuron bound broadcast via K=3 bf16 matmuls
           ones[3,128]^T @ split[3,512] -> PSUM; the rows are a hi/mid/lo
           bf16 split of the f32 vector, reconstructed exactly by the f32
           PSUM accumulation.  No DMA traffic.
    ACT  : PSUM->SBUF broadcast copies + a = |d| (Abs) in place.
    Pool : g = x - xshift (all chunks) + d = g + (-mean_b) for POOL_D chunks.
    DVE  : d for the rest, m = (|d| <= ks_b), y = m * x.

d-completion uses two sems (DD: DVE, DP: Pool) so each stays monotonic in
chunk order.  Engine busy (cost model): DMA 93.7us, DVE ~94.7, Pool ~93.6,
ACT 50, PE 13.7 -- right at the 16+16 MiB HBM roofline.  Same-engine dep
pairs rely on in-order engine execution (HW auto-drains between ops);
drains=True adds explicit drains for CoreSim's conservative race detector.
"""

import numpy as np

import concourse.bass as bass
import concourse.mybir as mybir

B, N = 4096, 8192
N_CORES = 8
ROWS = B // N_CORES   # 512 rows per core
P = 128
NT = ROWS // P        # 4 row tiles
CHUNK = 1024
NCH = N // CHUNK      # 8 chunks per row tile
NIDX = NT * NCH       # 32 chunk-steps per core
R = 512               # PSUM broadcast range (one bank)
Q = 2048              # load-quarter width
# chunks whose d runs on Pool, per row tile (rest on DVE); tile 0 is
# lighter on Pool so the pipeline ramp is not Pool-paced
POOL_D = {0: (6, 7), 1: (2, 5, 7), 2: (2, 5, 7), 3: (2, 5, 7)}

f32 = mybir.dt.float32
bf16 = mybir.dt.bfloat16


def build_nc(pool_d=POOL_D, drains=True):
    sub = mybir.AluOpType.subtract
    add = mybir.AluOpType.add
    mult = mybir.AluOpType.mult
    is_le = mybir.AluOpType.is_le
    Abs = mybir.ActivationFunctionType.Abs
    Copy = mybir.ActivationFunctionType.Copy

    nc = bass.Bass(detect_race_conditions=drains)
    x = nc.dram_tensor("x", [ROWS, N], f32, kind="ExternalInput")
    # vecd: [3, 2N+128] bf16 = hi/mid/lo splits of -mean | k*sqrt(var) | ones
    vecd = nc.dram_tensor("vecd", [3, 2 * N + P], bf16, kind="ExternalInput")
    y = nc.dram_tensor("y", [ROWS, N], f32, kind="ExternalOutput")

    if isinstance(pool_d, dict):
        pd_set = {
            t * NCH + c for t in range(NT) for c in pool_d.get(t, ()) }
    else:
        pd_set = {i for i in range(NIDX) if i % NCH in pool_d}

    def ndd(idx):
        """DVE-computed d's with id <= idx."""
        return sum(1 for i in range(idx + 1) if i not in pd_set)

    def ndp(idx):
        return sum(1 for i in range(idx + 1) if i in pd_set)

    from contextlib import ExitStack

    with ExitStack() as ctx:
        sb = lambda name, shape, dt=f32: ctx.enter_context(
            nc.sbuf_tensor(name, shape, dt)
        )
        xt = [sb(f"xt{i}", [P, N + 1]) for i in range(2)]
        mean_b = sb("mean_b", [P, N])   # holds -mean (host negates)
        ks_b = sb("ks_b", [P, N])
        vec = sb("vec", [3, 2 * N + P], bf16)
        msp = vec[:, 0:N]
        ksp = vec[:, N : 2 * N]
        ones = vec[:, 2 * N : 2 * N + P]
        gb = [sb(f"g{i}", [P, CHUNK]) for i in range(3)]
        db = [sb(f"d{i}", [P, CHUNK]) for i in range(2)]   # d then |d| in place
        mb = [sb(f"m{i}", [P, CHUNK]) for i in range(2)]
        ym = [sb(f"ym{i}", [P, CHUNK]) for i in range(4)]
        ps = [ctx.enter_context(nc.psum_tensor(f"ps{i}", [P, 2 * R], f32))
              for i in range(2)]

        sem = lambda name: ctx.enter_context(nc.semaphore(name))
        LV = sem("LV")       # vec load (1 DMA x16)
        E0 = sem("E0")       # tile-0 wrap + first eighth (2 DMAs x16)
        LQ = [[sem(f"LQ{s}_{q}") for q in range(4)] for s in range(2)]
        BB = sem("BB")       # PE matmul done (per matmul)
        C = sem("C")         # ACT bcast pair copy done (per 1024-range pair)
        PG = sem("PG")       # Pool g done (per chunk)
        DD = sem("DD")       # DVE d done (count of DVE-d's)
        DP = sem("DP")       # Pool d done (count of Pool-d's)
        A = sem("A")         # ACT |d| done (per chunk)
        Mm = sem("Mm")       # DVE m done (per chunk)
        V = sem("V")         # DVE y done (per chunk)
        S = [sem(f"S{i}") for i in range(4)]   # stores (x16)

        block = ctx.enter_context(nc.Block())

        # ---- load planning -------------------------------------------------
        def tile_plan(t):
            s = t % 2
            if t == 0:
                return [
                    ("wrap", E0), (0, CHUNK, E0),
                    (CHUNK, Q, LQ[s][0]),
                    (Q, 2 * Q, LQ[s][1]),
                    (2 * Q, 3 * Q, LQ[s][2]),
                    (3 * Q, 4 * Q, LQ[s][3]),
                ]
            return [
                ("wrap", LQ[s][0]), (0, Q, LQ[s][0]),
                (Q, 2 * Q, LQ[s][1]),
                (2 * Q, 3 * Q, LQ[s][2]),
                (3 * Q, 4 * Q, LQ[s][3]),
            ]

        plans = {t: tile_plan(t) for t in range(NT)}

        # g(t, c) reads x columns [c*CHUNK-1, (c+1)*CHUNK) (wrap for c == 0)
        sem_count = {}
        g_waits = {}
        for t in range(NT):
            seg_done = []
            for seg in plans[t]:
                semh = seg[-1]
                sem_count[id(semh)] = sem_count.get(id(semh), 0) + 16
                cs, ce = (-1, 0) if seg[0] == "wrap" else (seg[0], seg[1])
                seg_done.append((cs, ce, semh, sem_count[id(semh)]))
            for c in range(NCH):
                lo = c * CHUNK - 1
                hi = (c + 1) * CHUNK
                waits = {}
                for cs, ce, semh, cnt in seg_done:
                    if cs < hi and ce > lo:
                        key = id(semh)
                        if key not in waits or waits[key][1] < cnt:
                            waits[key] = (semh, cnt)
                g_waits[(t, c)] = list(waits.values())

        @block.sync
        def _(sync):
            def emit_loads(t, segs):
                rows = x[t * P : (t + 1) * P]
                s = t % 2
                for seg in segs:
                    semh = seg[-1]
                    if seg[0] == "wrap":
                        with nc.allow_non_contiguous_dma(reason="wrap col"):
                            sync.dma_start(
                                out=xt[s][:, 0:1], in_=rows[:, N - 1 : N]
                            ).then_inc(semh, 16)
                    else:
                        cs, ce = seg[0], seg[1]
                        sync.dma_start(
                            out=xt[s][:, 1 + cs : 1 + ce], in_=rows[:, cs:ce]
                        ).then_inc(semh, 16)

            emit_loads(0, plans[0][:2])       # wrap + first eighth
            sync.dma_start(out=vec[:], in_=vecd[:]).then_inc(LV, 16)
            emit_loads(0, plans[0][2:])
            emit_loads(1, plans[1])
            for idx in range(NIDX):
                t, c = divmod(idx, NCH)
                sync.wait_ge(V, idx + 1)
                sync.dma_start(
                    out=y[t * P : (t + 1) * P, c * CHUNK : (c + 1) * CHUNK],
                    in_=ym[idx % 4][:],
                ).then_inc(S[idx % 4], 16)
                # tile t+2 loads stream in as slot quarters free up:
                # store (t, 2q+2)'s V-wait implies y(t, 2q+2) done.
                if t + 2 < NT and c in (2, 4, 6, 7):
                    qi = {2: 0, 4: 1, 6: 2, 7: 3}[c]
                    segs = plans[t + 2]
                    if qi == 0:
                        emit_loads(t + 2, segs[:2])
                    else:
                        emit_loads(t + 2, segs[qi + 1 : qi + 2])

        @block.tensor
        def _(tensor):
            # broadcast pairs: p = 2*rr + (0: -mean, 1: ks), rr a 1024-range
            tensor.wait_ge(LV, 16)
            for p in range(2 * NCH):
                rr, which = divmod(p, 2)
                src = msp if which == 0 else ksp
                if p >= 2:
                    tensor.wait_ge(C, p - 1)  # ACT copied ps[p%2], reusable
                for h in range(2):
                    r0 = rr * CHUNK + h * R
                    tensor.matmul(
                        ps[p % 2][:, h * R : (h + 1) * R],
                        ones,
                        src[:, r0 : r0 + R],
                        start=True,
                        stop=True,
                    ).then_inc(BB, 1)

        @block.scalar
        def _(scalar):
            q = 0

            def copies(k):
                nonlocal q
                for _ in range(k):
                    rr, which = divmod(q, 2)
                    dst = mean_b if which == 0 else ks_b
                    scalar.wait_ge(BB, 2 * q + 2)  # both halves of pair q
                    scalar.activation(
                        dst[:, rr * CHUNK : (rr + 1) * CHUNK], ps[q % 2][:], Copy
                    ).then_inc(C, 1)
                    q += 1

            for idx in range(NIDX):
                if q < 2 * NCH:
                    copies(2)
                if idx in pd_set:
                    scalar.wait_ge(DP, ndp(idx))
                else:
                    scalar.wait_ge(DD, ndd(idx))
                # |d| in place: db[idx%2] both source and destination
                scalar.activation(db[idx % 2][:], db[idx % 2][:], Abs).then_inc(A, 1)

        @block.gpsimd
        def _(gpsimd):
            # step i: g(i), then d(i-1) when (i-1) is a Pool-d chunk
            for i in range(NIDX + 1):
                if i < NIDX:
                    t, c = divmod(i, NCH)
                    for semh, thresh in g_waits[(t, c)]:
                        gpsimd.wait_ge(semh, thresh)
                    if i >= 3 and (i - 3) not in pd_set:
                        gpsimd.wait_ge(DD, ndd(i - 3))  # gb[i%3] free
                    # (i-3) in pd_set: Pool's own d(i-3) precedes in order
                    if drains and i >= 3 and (i - 3) in pd_set:
                        gpsimd.drain()  # WAR: own d(i-3) read gb[i%3]
                    c0 = c * CHUNK
                    gpsimd.tensor_tensor(
                        gb[i % 3][:],
                        xt[t % 2][:, c0 + 1 : c0 + CHUNK + 1],
                        xt[t % 2][:, c0 : c0 + CHUNK],
                        sub,
                    ).then_inc(PG, 1)
                j = i - 1
                if 0 <= j < NIDX and j in pd_set:
                    tj, cj = divmod(j, NCH)
                    cj0 = cj * CHUNK
                    gpsimd.wait_ge(C, 2 * cj + 1)
                    if j >= 2:
                        gpsimd.wait_ge(Mm, j - 1)  # db[j%2] free (m(j-2) done)
                    if drains:
                        gpsimd.drain()  # RAW: reads gb[j%3] from own g(j)
                    # mean_b holds -mean, so d = g + mean_b
                    gpsimd.tensor_tensor(
                        db[j % 2][:], gb[j % 3][:],
                        mean_b[:, cj0 : cj0 + CHUNK], add,
                    ).then_inc(DP, 1)

        @block.vector
        def _(vector):
            # step i: m(i-2), d(i), y(i-3)   (m before d: db[i%2] WAR)
            for i in range(NIDX + 3):
                j = i - 2
                if 0 <= j < NIDX:
                    tj, cj = divmod(j, NCH)
                    vector.wait_ge(A, j + 1)
                    vector.wait_ge(C, 2 * cj + 2)
                    if drains:
                        vector.drain()  # mb[j%2] WAR vs y(j-2); db read
                    vector.tensor_tensor(
                        mb[j % 2][:],
                        db[j % 2][:],
                        ks_b[:, cj * CHUNK : (cj + 1) * CHUNK],
                        is_le,
                    ).then_inc(Mm, 1)
                if i < NIDX and i not in pd_set:
                    t, c = divmod(i, NCH)
                    c0 = c * CHUNK
                    vector.wait_ge(PG, i + 1)
                    vector.wait_ge(C, 2 * c + 1)
                    if i >= 3 and (i - 3) in pd_set:
                        vector.wait_ge(DP, ndp(i - 3))  # gb[i%3] free
                    # (i-3) DVE-d: own order.  db[i%2] free: m(i-2) precedes.
                    if drains:
                        vector.drain()
                    vector.tensor_tensor(
                        db[i % 2][:], gb[i % 3][:], mean_b[:, c0 : c0 + CHUNK], add
                    ).then_inc(DD, 1)
                jy = i - 3
                if 0 <= jy < NIDX:
                    ty, cy = divmod(jy, NCH)
                    cy0 = cy * CHUNK
                    vector.wait_ge(Mm, jy + 1)
                    if jy >= 4:
                        vector.wait_ge(S[jy % 4], 16 * (jy // 4))  # ym free
                    if drains:
                        vector.drain()
                    vector.tensor_tensor(
                        ym[jy % 4][:],
                        mb[jy % 2][:],
                        xt[ty % 2][:, cy0 + 1 : cy0 + CHUNK + 1],
                        mult,
                    ).then_inc(V, 1)

    return nc


def _host_vectors(mean_grad, var_grad, k):
    import ml_dtypes

    mg = np.asarray(mean_grad, dtype=np.float32)
    vg = np.asarray(var_grad, dtype=np.float32)
    kf = np.float32(k)
    ks = (kf * np.sqrt(vg, dtype=np.float32)).astype(np.float32)

    def split3(v):
        hi = v.astype(ml_dtypes.bfloat16)
        r1 = v - hi.astype(np.float32)
        mid = r1.astype(ml_dtypes.bfloat16)
        r2 = r1 - mid.astype(np.float32)
        lo = r2.astype(ml_dtypes.bfloat16)
        return np.stack([hi, mid, lo])

    vec = np.empty((3, 2 * N + P), dtype=ml_dtypes.bfloat16)
    vec[:, 0:N] = split3(-mg)
    vec[:, N : 2 * N] = split3(ks)
    vec[:, 2 * N :] = np.ones((3, P), dtype=ml_dtypes.bfloat16)
    return vec


class _FastRunner:
    """Cached PJRT dispatch (axon path).

    run_bass_kernel_spmd -> run_bass_via_pjrt rebuilds jax.jit(shard_map(...))
    every call (retrace), transfers 128 MiB of host zeros for the donated
    outputs, and splits/reconcatenates the output.  This does the lowering
    once, keeps the compiled callable, creates the donated zeros on device,
    and feeds the full [4096, 8192] input directly.
    """

    def __init__(self, nc, n_cores):
        import jax
        import jax.numpy as jnp
        from jax.sharding import Mesh, NamedSharding, PartitionSpec
        from jax.experimental.shard_map import shard_map
        from concourse import bass2jax
        import concourse.mybir as mybir

        bass2jax.install_neuronx_cc_hook()
        in_names = []
        out_names = []
        out_avals = []
        zero_shapes = []
        partition_name = (
            nc.partition_id_tensor.name if nc.partition_id_tensor else None
        )
        for alloc in nc.m.functions[0].allocations:
            if not isinstance(alloc, mybir.MemoryLocationSet):
                continue
            name = alloc.memorylocations[0].name
            if alloc.kind == "ExternalInput":
                if name != partition_name:
                    in_names.append(name)
            elif alloc.kind == "ExternalOutput":
                shape = tuple(alloc.tensor_shape)
                dtype = mybir.dt.np(alloc.dtype)
                out_names.append(name)
                out_avals.append(jax.core.ShapedArray(shape, dtype))
                zero_shapes.append((shape, dtype))
        if nc.dbg_addr is not None:
            raise RuntimeError("debug nc unsupported in fast path")
        self.in_names = in_names
        n_params = len(in_names)
        n_outs = len(out_names)
        all_in_names = list(in_names) + list(out_names)
        if partition_name is not None:
            all_in_names.append(partition_name)

        def _body(*args):
            operands = list(args)
            if partition_name is not None:
                operands.append(bass2jax.partition_id_tensor())
            outs = bass2jax._bass_exec_p.bind(
                *operands,
                out_avals=tuple(out_avals),
                in_names=tuple(all_in_names),
                out_names=tuple(out_names),
                lowering_input_output_aliases=(),
                sim_require_finite=True,
                sim_require_nnan=True,
                nc=nc,
            )
            return tuple(outs)

        devices = jax.devices()[:n_cores]
        assert len(devices) == n_cores, len(jax.devices())
        mesh = Mesh(np.asarray(devices), ("core",))
        spec = PartitionSpec("core")
        self._sharded = jax.jit(
            shard_map(
                _body,
                mesh=mesh,
                in_specs=(spec,) * (n_params + n_outs),
                out_specs=(spec,) * n_outs,
                check_rep=False,
            ),
            donate_argnums=tuple(range(n_params, n_params + n_outs)),
            keep_unused=True,
        )
        sharding = NamedSharding(mesh, spec)
        self._make_zeros = jax.jit(
            lambda: tuple(
                jnp.zeros((n_cores * s[0], *s[1:]), d) for s, d in zero_shapes
            ),
            out_shardings=(sharding,) * n_outs,
        )

    def __call__(self, *global_inputs):
        zeros = self._make_zeros()
        outs = self._sharded(*global_inputs, *zeros)
        return [np.asarray(o) for o in outs]


_CACHE = {}


def _run_fallback(nc, x, vec):
    from concourse.bass_utils import run_bass_kernel_spmd

    in_maps = [
        {"x": x[i * ROWS : (i + 1) * ROWS], "vecd": vec} for i in range(N_CORES)
    ]
    res = run_bass_kernel_spmd(nc, in_maps, core_ids=list(range(N_CORES)))
    return np.concatenate([res.results[i]["y"] for i in range(N_CORES)], axis=0)


def kernel(output, mean_grad, var_grad, k):
    x = np.ascontiguousarray(np.asarray(output, dtype=np.float32))
    assert x.shape == (B, N), x.shape
    vec = _host_vectors(mean_grad, var_grad, k)

    if "nc" not in _CACHE:
        _CACHE["nc"] = build_nc(drains=False)
    nc = _CACHE["nc"]

    try:
        if "runner" not in _CACHE:
            _CACHE["runner"] = _FastRunner(nc, N_CORES)
        runner = _CACHE["runner"]
        vec8 = np.ascontiguousarray(np.tile(vec, (N_CORES, 1)))
        ins = {"x": x, "vecd": vec8}
        outs = runner(*[ins[nm] for nm in runner.in_names])
        return outs[0]
    except Exception:
        _CACHE.pop("runner", None)
        return _run_fallback(nc, x, vec)


# revision 5
# speedup vs baseline: 1.0330x; 1.0330x over previous
"""Trainium2 Bass kernel for nn_Correction_Module_dense.

Math (equivalent to the jax reference):
    g    = x - roll(x, 1, axis=1)            # circular diff along neuron axis
    mask = |g - mean_grad| <= k*sqrt(var_grad)
    y    = x * mask

Sharding: pure data parallel over batch; 8 cores x [512, 8192] slabs.

Per-core pipeline, [128, 1024] chunks (32 chunk-steps).  GPSIMD's walrus
codegen only accepts add/subtract/mult tensor_tensor, so:
    SP   : all DMAs (quarter-granularity x loads; tile-0's first quarter is
           split so compute starts ~5 us in).  xt column 0 holds x[:, N-1]
           (wrap), making g a uniform shifted-AP subtract.
    PE   : per-neuron bound broadcast via K=3 bf16 matmuls
           ones[3,128]^T @ split[3,512] -> PSUM; the rows are a hi/mid/lo
           bf16 split of the f32 vector, reconstructed exactly by the f32
           PSUM accumulation.  No DMA traffic.
    ACT  : PSUM->SBUF broadcast copies + a = |d| (Abs) in place.
    Pool : g = x - xshift (all chunks) + d = g + (-mean_b) for POOL_D chunks.
    DVE  : d for the rest, m = (|d| <= ks_b), y = m * x.

d-completion uses two sems (DD: DVE, DP: Pool) so each stays monotonic in
chunk order.  Engine busy (cost model): DMA 93.7us, DVE ~94.7, Pool ~93.6,
ACT 50, PE 13.7 -- right at the 16+16 MiB HBM roofline.  Same-engine dep
pairs rely on in-order engine execution (HW auto-drains between ops);
drains=True adds explicit drains for CoreSim's conservative race detector.
"""

import numpy as np

import concourse.bass as bass
import concourse.mybir as mybir

B, N = 4096, 8192
N_CORES = 8
ROWS = B // N_CORES   # 512 rows per core
P = 128
NT = ROWS // P        # 4 row tiles
CHUNK = 1024
NCH = N // CHUNK      # 8 chunks per row tile
NIDX = NT * NCH       # 32 chunk-steps per core
R = 512               # PSUM broadcast range (one bank)
Q = 2048              # load-quarter width
# chunks whose d runs on Pool, per row tile (rest on DVE); tile 0 is
# lighter on Pool so the pipeline ramp is not Pool-paced
POOL_D = {0: (6, 7), 1: (2, 5, 7), 2: (2, 5, 7), 3: (2, 5, 7)}
# chunk-steps whose g runs on DVE instead of Pool (none: measured neutral --
# the pipeline is not start-bound -- but the machinery is kept for tuning)
DVE_G = ()

f32 = mybir.dt.float32
bf16 = mybir.dt.bfloat16


def build_nc(pool_d=POOL_D, dve_g=DVE_G, drains=True):
    sub = mybir.AluOpType.subtract
    add = mybir.AluOpType.add
    mult = mybir.AluOpType.mult
    is_le = mybir.AluOpType.is_le
    Abs = mybir.ActivationFunctionType.Abs
    Copy = mybir.ActivationFunctionType.Copy

    nc = bass.Bass(detect_race_conditions=drains)
    x = nc.dram_tensor("x", [ROWS, N], f32, kind="ExternalInput")
    # vecd: [3, 2N+128] bf16 = hi/mid/lo splits of -mean | k*sqrt(var) | ones
    vecd = nc.dram_tensor("vecd", [3, 2 * N + P], bf16, kind="ExternalInput")
    y = nc.dram_tensor("y", [ROWS, N], f32, kind="ExternalOutput")

    if isinstance(pool_d, dict):
        pd_set = {
            t * NCH + c for t in range(NT) for c in pool_d.get(t, ()) }
    else:
        pd_set = {i for i in range(NIDX) if i % NCH in pool_d}

    def ndd(idx):
        """DVE-computed d's with id <= idx."""
        return sum(1 for i in range(idx + 1) if i not in pd_set)

    def ndp(idx):
        return sum(1 for i in range(idx + 1) if i in pd_set)

    from contextlib import ExitStack

    with ExitStack() as ctx:
        sb = lambda name, shape, dt=f32: ctx.enter_context(
            nc.sbuf_tensor(name, shape, dt)
        )
        xt = [sb(f"xt{i}", [P, N + 1]) for i in range(2)]
        mean_b = sb("mean_b", [P, N])   # holds -mean (host negates)
        ks_b = sb("ks_b", [P, N])
        vec = sb("vec", [3, 2 * N + P], bf16)
        msp = vec[:, 0:N]
        ksp = vec[:, N : 2 * N]
        ones = vec[:, 2 * N : 2 * N + P]
        gb = [sb(f"g{i}", [P, CHUNK]) for i in range(3)]
        db = [sb(f"d{i}", [P, CHUNK]) for i in range(2)]   # d then |d| in place
        mb = [sb(f"m{i}", [P, CHUNK]) for i in range(2)]
        ym = [sb(f"ym{i}", [P, CHUNK]) for i in range(4)]
        ps = [ctx.enter_context(nc.psum_tensor(f"ps{i}", [P, 2 * R], f32))
              for i in range(2)]

        sem = lambda name: ctx.enter_context(nc.semaphore(name))
        LV = sem("LV")       # vec load (1 DMA x16)
        E0 = sem("E0")       # tile-0 wrap + first eighth (2 DMAs x16)
        LQ = [[sem(f"LQ{s}_{q}") for q in range(4)] for s in range(2)]
        BB = sem("BB")       # PE matmul done (per matmul)
        C = sem("C")         # ACT bcast pair copy done (per 1024-range pair)
        PG = sem("PG")       # Pool g done (per chunk)
        DD = sem("DD")       # DVE d done (count of DVE-d's)
        DP = sem("DP")       # Pool d done (count of Pool-d's)
        A = sem("A")         # ACT |d| done (per chunk)
        Mm = sem("Mm")       # DVE m done (per chunk)
        V = sem("V")         # DVE y done (per chunk)
        S = [sem(f"S{i}") for i in range(4)]   # stores (x16)

        dg_set = set(dve_g)

        def pg_count(idx):
            return sum(1 for i in range(idx + 1) if i not in dg_set)

        block = ctx.enter_context(nc.Block())

        # ---- load planning -------------------------------------------------
        def tile_plan(t):
            s = t % 2
            if t == 0:
                return [
                    ("wrap", E0), (0, CHUNK, E0),
                    (CHUNK, Q, LQ[s][0]),
                    (Q, 2 * Q, LQ[s][1]),
                    (2 * Q, 3 * Q, LQ[s][2]),
                    (3 * Q, 4 * Q, LQ[s][3]),
                ]
            return [
                ("wrap", LQ[s][0]), (0, Q, LQ[s][0]),
                (Q, 2 * Q, LQ[s][1]),
                (2 * Q, 3 * Q, LQ[s][2]),
                (3 * Q, 4 * Q, LQ[s][3]),
            ]

        plans = {t: tile_plan(t) for t in range(NT)}

        # g(t, c) reads x columns [c*CHUNK-1, (c+1)*CHUNK) (wrap for c == 0)
        sem_count = {}
        g_waits = {}
        for t in range(NT):
            seg_done = []
            for seg in plans[t]:
                semh = seg[-1]
                sem_count[id(semh)] = sem_count.get(id(semh), 0) + 16
                cs, ce = (-1, 0) if seg[0] == "wrap" else (seg[0], seg[1])
                seg_done.append((cs, ce, semh, sem_count[id(semh)]))
            for c in range(NCH):
                lo = c * CHUNK - 1
                hi = (c + 1) * CHUNK
                waits = {}
                for cs, ce, semh, cnt in seg_done:
                    if cs < hi and ce > lo:
                        key = id(semh)
                        if key not in waits or waits[key][1] < cnt:
                            waits[key] = (semh, cnt)
                g_waits[(t, c)] = list(waits.values())

        @block.sync
        def _(sync):
            def emit_loads(t, segs):
                rows = x[t * P : (t + 1) * P]
                s = t % 2
                for seg in segs:
                    semh = seg[-1]
                    if seg[0] == "wrap":
                        with nc.allow_non_contiguous_dma(reason="wrap col"):
                            sync.dma_start(
                                out=xt[s][:, 0:1], in_=rows[:, N - 1 : N]
                            ).then_inc(semh, 16)
                    else:
                        cs, ce = seg[0], seg[1]
                        sync.dma_start(
                            out=xt[s][:, 1 + cs : 1 + ce], in_=rows[:, cs:ce]
                        ).then_inc(semh, 16)

            emit_loads(0, plans[0][:2])       # wrap + first eighth
            sync.dma_start(out=vec[:], in_=vecd[:]).then_inc(LV, 16)
            emit_loads(0, plans[0][2:])
            emit_loads(1, plans[1])
            for idx in range(NIDX):
                t, c = divmod(idx, NCH)
                sync.wait_ge(V, idx + 1)
                sync.dma_start(
                    out=y[t * P : (t + 1) * P, c * CHUNK : (c + 1) * CHUNK],
                    in_=ym[idx % 4][:],
                ).then_inc(S[idx % 4], 16)
                # tile t+2 loads stream in as slot quarters free up:
                # store (t, 2q+2)'s V-wait implies y(t, 2q+2) done.
                if t + 2 < NT and c in (2, 4, 6, 7):
                    qi = {2: 0, 4: 1, 6: 2, 7: 3}[c]
                    segs = plans[t + 2]
                    if qi == 0:
                        emit_loads(t + 2, segs[:2])
                    else:
                        emit_loads(t + 2, segs[qi + 1 : qi + 2])

        @block.tensor
        def _(tensor):
            # broadcast pairs: p = 2*rr + (0: -mean, 1: ks), rr a 1024-range
            tensor.wait_ge(LV, 16)
            for p in range(2 * NCH):
                rr, which = divmod(p, 2)
                src = msp if which == 0 else ksp
                if p >= 2:
                    tensor.wait_ge(C, p - 1)  # ACT copied ps[p%2], reusable
                for h in range(2):
                    r0 = rr * CHUNK + h * R
                    tensor.matmul(
                        ps[p % 2][:, h * R : (h + 1) * R],
                        ones,
                        src[:, r0 : r0 + R],
                        start=True,
                        stop=True,
                    ).then_inc(BB, 1)

        @block.scalar
        def _(scalar):
            q = 0

            def copies(k):
                nonlocal q
                for _ in range(k):
                    rr, which = divmod(q, 2)
                    dst = mean_b if which == 0 else ks_b
                    scalar.wait_ge(BB, 2 * q + 2)  # both halves of pair q
                    scalar.activation(
                        dst[:, rr * CHUNK : (rr + 1) * CHUNK], ps[q % 2][:], Copy
                    ).then_inc(C, 1)
                    q += 1

            for idx in range(NIDX):
                if q < 2 * NCH:
                    copies(2)
                if idx in pd_set:
                    scalar.wait_ge(DP, ndp(idx))
                else:
                    scalar.wait_ge(DD, ndd(idx))
                # |d| in place: db[idx%2] both source and destination
                scalar.activation(db[idx % 2][:], db[idx % 2][:], Abs).then_inc(A, 1)

        @block.gpsimd
        def _(gpsimd):
            # step i: g(i), then d(i-1) when (i-1) is a Pool-d chunk
            for i in range(NIDX + 1):
                if i < NIDX and i not in dg_set:
                    t, c = divmod(i, NCH)
                    for semh, thresh in g_waits[(t, c)]:
                        gpsimd.wait_ge(semh, thresh)
                    if i >= 3 and (i - 3) not in pd_set:
                        gpsimd.wait_ge(DD, ndd(i - 3))  # gb[i%3] free
                    # (i-3) in pd_set: Pool's own d(i-3) precedes in order
                    if drains and i >= 3 and (i - 3) in pd_set:
                        gpsimd.drain()  # WAR: own d(i-3) read gb[i%3]
                    c0 = c * CHUNK
                    gpsimd.tensor_tensor(
                        gb[i % 3][:],
                        xt[t % 2][:, c0 + 1 : c0 + CHUNK + 1],
                        xt[t % 2][:, c0 : c0 + CHUNK],
                        sub,
                    ).then_inc(PG, 1)
                j = i - 1
                if 0 <= j < NIDX and j in pd_set:
                    tj, cj = divmod(j, NCH)
                    cj0 = cj * CHUNK
                    gpsimd.wait_ge(C, 2 * cj + 1)
                    if j >= 2:
                        gpsimd.wait_ge(Mm, j - 1)  # db[j%2] free (m(j-2) done)
                    if drains:
                        gpsimd.drain()  # RAW: reads gb[j%3] from own g(j)
                    # mean_b holds -mean, so d = g + mean_b
                    gpsimd.tensor_tensor(
                        db[j % 2][:], gb[j % 3][:],
                        mean_b[:, cj0 : cj0 + CHUNK], add,
                    ).then_inc(DP, 1)

        @block.vector
        def _(vector):
            # step i: m(i-2), d(i), y(i-3)   (m before d: db[i%2] WAR)
            for i in range(NIDX + 3):
                j = i - 2
                if 0 <= j < NIDX:
                    tj, cj = divmod(j, NCH)
                    vector.wait_ge(A, j + 1)
                    vector.wait_ge(C, 2 * cj + 2)
                    if drains:
                        vector.drain()  # mb[j%2] WAR vs y(j-2); db read
                    vector.tensor_tensor(
                        mb[j % 2][:],
                        db[j % 2][:],
                        ks_b[:, cj * CHUNK : (cj + 1) * CHUNK],
                        is_le,
                    ).then_inc(Mm, 1)
                if i < NIDX and i in dg_set:
                    t, c = divmod(i, NCH)
                    c0 = c * CHUNK
                    for semh, thresh in g_waits[(t, c)]:
                        vector.wait_ge(semh, thresh)
                    if drains:
                        vector.drain()
                    vector.tensor_tensor(
                        gb[i % 3][:],
                        xt[t % 2][:, c0 + 1 : c0 + CHUNK + 1],
                        xt[t % 2][:, c0 : c0 + CHUNK],
                        sub,
                    )
                if i < NIDX and i not in pd_set:
                    t, c = divmod(i, NCH)
                    c0 = c * CHUNK
                    if i not in dg_set:
                        vector.wait_ge(PG, pg_count(i))
                    vector.wait_ge(C, 2 * c + 1)
                    if i >= 3 and (i - 3) in pd_set:
                        vector.wait_ge(DP, ndp(i - 3))  # gb[i%3] free
                    # (i-3) DVE-d: own order.  db[i%2] free: m(i-2) precedes.
                    if drains:
                        vector.drain()
                    vector.tensor_tensor(
                        db[i % 2][:], gb[i % 3][:], mean_b[:, c0 : c0 + CHUNK], add
                    ).then_inc(DD, 1)
                jy = i - 3
                if 0 <= jy < NIDX:
                    ty, cy = divmod(jy, NCH)
                    cy0 = cy * CHUNK
                    vector.wait_ge(Mm, jy + 1)
                    if jy >= 4:
                        vector.wait_ge(S[jy % 4], 16 * (jy // 4))  # ym free
                    if drains:
                        vector.drain()
                    vector.tensor_tensor(
                        ym[jy % 4][:],
                        mb[jy % 2][:],
                        xt[ty % 2][:, cy0 + 1 : cy0 + CHUNK + 1],
                        mult,
                    ).then_inc(V, 1)

    return nc


def _host_vectors(mean_grad, var_grad, k):
    import ml_dtypes

    mg = np.asarray(mean_grad, dtype=np.float32)
    vg = np.asarray(var_grad, dtype=np.float32)
    kf = np.float32(k)
    ks = (kf * np.sqrt(vg, dtype=np.float32)).astype(np.float32)

    def split3(v):
        hi = v.astype(ml_dtypes.bfloat16)
        r1 = v - hi.astype(np.float32)
        mid = r1.astype(ml_dtypes.bfloat16)
        r2 = r1 - mid.astype(np.float32)
        lo = r2.astype(ml_dtypes.bfloat16)
        return np.stack([hi, mid, lo])

    vec = np.empty((3, 2 * N + P), dtype=ml_dtypes.bfloat16)
    vec[:, 0:N] = split3(-mg)
    vec[:, N : 2 * N] = split3(ks)
    vec[:, 2 * N :] = np.ones((3, P), dtype=ml_dtypes.bfloat16)
    return vec


class _FastRunner:
    """Cached PJRT dispatch (axon path).

    run_bass_kernel_spmd -> run_bass_via_pjrt rebuilds jax.jit(shard_map(...))
    every call (retrace), transfers 128 MiB of host zeros for the donated
    outputs, and splits/reconcatenates the output.  This does the lowering
    once, keeps the compiled callable, creates the donated zeros on device,
    and feeds the full [4096, 8192] input directly.
    """

    def __init__(self, nc, n_cores):
        import jax
        import jax.numpy as jnp
        from jax.sharding import Mesh, NamedSharding, PartitionSpec
        from jax.experimental.shard_map import shard_map
        from concourse import bass2jax
        import concourse.mybir as mybir

        bass2jax.install_neuronx_cc_hook()
        in_names = []
        out_names = []
        out_avals = []
        zero_shapes = []
        partition_name = (
            nc.partition_id_tensor.name if nc.partition_id_tensor else None
        )
        for alloc in nc.m.functions[0].allocations:
            if not isinstance(alloc, mybir.MemoryLocationSet):
                continue
            name = alloc.memorylocations[0].name
            if alloc.kind == "ExternalInput":
                if name != partition_name:
                    in_names.append(name)
            elif alloc.kind == "ExternalOutput":
                shape = tuple(alloc.tensor_shape)
                dtype = mybir.dt.np(alloc.dtype)
                out_names.append(name)
                out_avals.append(jax.core.ShapedArray(shape, dtype))
                zero_shapes.append((shape, dtype))
        if nc.dbg_addr is not None:
            raise RuntimeError("debug nc unsupported in fast path")
        self.in_names = in_names
        n_params = len(in_names)
        n_outs = len(out_names)
        all_in_names = list(in_names) + list(out_names)
        if partition_name is not None:
            all_in_names.append(partition_name)

        def _body(*args):
            operands = list(args)
            if partition_name is not None:
                operands.append(bass2jax.partition_id_tensor())
            outs = bass2jax._bass_exec_p.bind(
                *operands,
                out_avals=tuple(out_avals),
                in_names=tuple(all_in_names),
                out_names=tuple(out_names),
                lowering_input_output_aliases=(),
                sim_require_finite=True,
                sim_require_nnan=True,
                nc=nc,
            )
            return tuple(outs)

        devices = jax.devices()[:n_cores]
        assert len(devices) == n_cores, len(jax.devices())
        mesh = Mesh(np.asarray(devices), ("core",))
        spec = PartitionSpec("core")
        self._sharded = jax.jit(
            shard_map(
                _body,
                mesh=mesh,
                in_specs=(spec,) * (n_params + n_outs),
                out_specs=(spec,) * n_outs,
                check_rep=False,
            ),
            donate_argnums=tuple(range(n_params, n_params + n_outs)),
            keep_unused=True,
        )
        sharding = NamedSharding(mesh, spec)
        self._make_zeros = jax.jit(
            lambda: tuple(
                jnp.zeros((n_cores * s[0], *s[1:]), d) for s, d in zero_shapes
            ),
            out_shardings=(sharding,) * n_outs,
        )

    def __call__(self, *global_inputs):
        zeros = self._make_zeros()
        outs = self._sharded(*global_inputs, *zeros)
        return [np.asarray(o) for o in outs]


_CACHE = {}


def _run_fallback(nc, x, vec):
    from concourse.bass_utils import run_bass_kernel_spmd

    in_maps = [
        {"x": x[i * ROWS : (i + 1) * ROWS], "vecd": vec} for i in range(N_CORES)
    ]
    res = run_bass_kernel_spmd(nc, in_maps, core_ids=list(range(N_CORES)))
    return np.concatenate([res.results[i]["y"] for i in range(N_CORES)], axis=0)


def kernel(output, mean_grad, var_grad, k):
    x = np.ascontiguousarray(np.asarray(output, dtype=np.float32))
    assert x.shape == (B, N), x.shape
    vec = _host_vectors(mean_grad, var_grad, k)

    if "nc" not in _CACHE:
        _CACHE["nc"] = build_nc(drains=False)
    nc = _CACHE["nc"]

    try:
        if "runner" not in _CACHE:
            _CACHE["runner"] = _FastRunner(nc, N_CORES)
        runner = _CACHE["runner"]
        vec8 = np.ascontiguousarray(np.tile(vec, (N_CORES, 1)))
        ins = {"x": x, "vecd": vec8}
        outs = runner(*[ins[nm] for nm in runner.in_names])
        return outs[0]
    except Exception:
        _CACHE.pop("runner", None)
        return _run_fallback(nc, x, vec)


# revision 6
# speedup vs baseline: 1.0544x; 1.0207x over previous
"""Trainium2 Bass kernel for nn_Correction_Module_dense.

Math (equivalent to the jax reference):
    g    = x - roll(x, 1, axis=1)            # circular diff along neuron axis
    mask = |g - mean_grad| <= k*sqrt(var_grad)
    y    = x * mask

Sharding: pure data parallel over batch; 8 cores x [512, 8192] slabs.

Per-core pipeline, [128, 1024] chunks (32 chunk-steps).  GPSIMD's walrus
codegen only accepts add/subtract/mult tensor_tensor, so:
    SP   : all DMAs (quarter-granularity x loads; tile-0's first quarter is
           split so compute starts ~5 us in).  xt column 0 holds x[:, N-1]
           (wrap), making g a uniform shifted-AP subtract.
    PE   : per-neuron bound broadcast via K=3 bf16 matmuls
           ones[3,128]^T @ split[3,512] -> PSUM; the rows are a hi/mid/lo
           bf16 split of the f32 vector, reconstructed exactly by the f32
           PSUM accumulation.  No DMA traffic.
    ACT  : PSUM->SBUF broadcast copies + a = |d| (Abs) in place.
    Pool : g = x - xshift (all chunks) + d = g + (-mean_b) for POOL_D chunks.
    DVE  : d for the rest, m = (|d| <= ks_b), y = m * x.

d-completion uses two sems (DD: DVE, DP: Pool) so each stays monotonic in
chunk order.  Engine busy (cost model): DMA 93.7us, DVE ~94.7, Pool ~93.6,
ACT 50, PE 13.7 -- right at the 16+16 MiB HBM roofline.  Same-engine dep
pairs rely on in-order engine execution (HW auto-drains between ops);
drains=True adds explicit drains for CoreSim's conservative race detector.
"""

import numpy as np

import concourse.bass as bass
import concourse.mybir as mybir

B, N = 4096, 8192
N_CORES = 8
ROWS = B // N_CORES   # 512 rows per core
P = 128
NT = ROWS // P        # 4 row tiles
CHUNK = 1024
NCH = N // CHUNK      # 8 chunks per row tile
NIDX = NT * NCH       # 32 chunk-steps per core
R = 512               # PSUM broadcast range (one bank)
Q = 2048              # load-quarter width
# chunks whose d runs on Pool, per row tile; tile 0 is lighter on Pool so
# the pipeline ramp is not Pool-paced
POOL_D = {0: (6, 7), 1: (0, 2, 3, 5, 7), 2: (0, 2, 5, 7), 3: (0, 2, 5, 7)}
# chunks whose d is computed on PE straight from xt (no g op at all):
# psd = I@x[shifted] + (-I)@x + ones3@(-mean splits), accumulated in PSUM.
# Tile 0 is excluded (PE does the bound broadcasts then).
PE_D = {0: (), 1: (1, 4, 6), 2: (1, 4, 6), 3: (1, 4, 6)}
# chunk-steps whose g runs on DVE instead of Pool (none: measured neutral --
# the pipeline is not start-bound -- but the machinery is kept for tuning)
DVE_G = ()

f32 = mybir.dt.float32
bf16 = mybir.dt.bfloat16


def build_nc(pool_d=POOL_D, pe_d=PE_D, dve_g=DVE_G, drains=True):
    sub = mybir.AluOpType.subtract
    add = mybir.AluOpType.add
    mult = mybir.AluOpType.mult
    is_le = mybir.AluOpType.is_le
    Abs = mybir.ActivationFunctionType.Abs
    Copy = mybir.ActivationFunctionType.Copy

    nc = bass.Bass(detect_race_conditions=drains)
    x = nc.dram_tensor("x", [ROWS, N], f32, kind="ExternalInput")
    # vecd: [3, 2N+128] bf16 = hi/mid/lo splits of -mean | k*sqrt(var) | ones
    vecd = nc.dram_tensor("vecd", [3, 2 * N + P], bf16, kind="ExternalInput")
    identd = nc.dram_tensor("identd", [P, P], f32, kind="ExternalInput")
    nidentd = nc.dram_tensor("nidentd", [P, P], f32, kind="ExternalInput")
    y = nc.dram_tensor("y", [ROWS, N], f32, kind="ExternalOutput")

    if isinstance(pool_d, dict):
        pd_set = {
            t * NCH + c for t in range(NT) for c in pool_d.get(t, ()) }
    else:
        pd_set = {i for i in range(NIDX) if i % NCH in pool_d}
    pe_set = {t * NCH + c for t in range(NT) for c in pe_d.get(t, ())}
    assert not (pd_set & pe_set)
    pe_list = sorted(pe_set)
    pe_rank = {i: r for r, i in enumerate(pe_list)}

    def ndd(idx):
        """DVE-computed d's with id <= idx."""
        return sum(1 for i in range(idx + 1) if i not in pd_set and i not in pe_set)

    def ndp(idx):
        return sum(1 for i in range(idx + 1) if i in pd_set)

    from contextlib import ExitStack

    with ExitStack() as ctx:
        sb = lambda name, shape, dt=f32: ctx.enter_context(
            nc.sbuf_tensor(name, shape, dt)
        )
        xt = [sb(f"xt{i}", [P, N + 1]) for i in range(2)]
        mean_b = sb("mean_b", [P, N])   # holds -mean (host negates)
        ks_b = sb("ks_b", [P, N])
        vec = sb("vec", [3, 2 * N + P], bf16)
        ident = sb("ident", [P, P])
        nident = sb("nident", [P, P])
        msp = vec[:, 0:N]
        ksp = vec[:, N : 2 * N]
        ones = vec[:, 2 * N : 2 * N + P]
        gb = [sb(f"g{i}", [P, CHUNK]) for i in range(3)]
        db = [sb(f"d{i}", [P, CHUNK]) for i in range(2)]   # d then |d| in place
        mb = [sb(f"m{i}", [P, CHUNK]) for i in range(2)]
        ym = [sb(f"ym{i}", [P, CHUNK]) for i in range(4)]
        ps = [ctx.enter_context(nc.psum_tensor(f"ps{i}", [P, 2 * R], f32))
              for i in range(2)]
        psd = [ctx.enter_context(nc.psum_tensor(f"psd{i}", [P, CHUNK], f32))
               for i in range(2)]

        sem = lambda name: ctx.enter_context(nc.semaphore(name))
        LV = sem("LV")       # vec load (1 DMA x16)
        E0 = sem("E0")       # tile-0 wrap + first eighth (2 DMAs x16)
        LQ = [[sem(f"LQ{s}_{q}") for q in range(4)] for s in range(2)]
        BB = sem("BB")       # PE matmul done (per matmul)
        C = sem("C")         # ACT bcast pair copy done (per 1024-range pair)
        PG = sem("PG")       # Pool g done (per chunk)
        DD = sem("DD")       # DVE d done (count of DVE-d's)
        DP = sem("DP")       # Pool d done (count of Pool-d's)
        A = sem("A")         # ACT |d| done (per chunk)
        Mm = sem("Mm")       # DVE m done (per chunk)
        V = sem("V")         # DVE y done (per chunk)
        S = [sem(f"S{i}") for i in range(4)]   # stores (x16)

        dg_set = set(dve_g)

        def pg_count(idx):
            return sum(
                1 for i in range(idx + 1) if i not in dg_set and i not in pe_set
            )

        def gb_release_wait(eng, i):
            # gb slot i%3 was last written by the previous non-PE g with the
            # same slot; wait for its consumer d to finish.
            j = i - 3
            while j >= 0 and j in pe_set:
                j -= 3
            if j < 0:
                return
            if j in pd_set:
                eng.wait_ge(DP, ndp(j))
            else:
                eng.wait_ge(DD, ndd(j))

        block = ctx.enter_context(nc.Block())

        # ---- load planning -------------------------------------------------
        def tile_plan(t):
            s = t % 2
            if t == 0:
                return [
                    ("wrap", E0), (0, CHUNK, E0),
                    (CHUNK, Q, LQ[s][0]),
                    (Q, 2 * Q, LQ[s][1]),
                    (2 * Q, 3 * Q, LQ[s][2]),
                    (3 * Q, 4 * Q, LQ[s][3]),
                ]
            return [
                ("wrap", LQ[s][0]), (0, Q, LQ[s][0]),
                (Q, 2 * Q, LQ[s][1]),
                (2 * Q, 3 * Q, LQ[s][2]),
                (3 * Q, 4 * Q, LQ[s][3]),
            ]

        plans = {t: tile_plan(t) for t in range(NT)}

        # g(t, c) reads x columns [c*CHUNK-1, (c+1)*CHUNK) (wrap for c == 0)
        sem_count = {}
        g_waits = {}
        for t in range(NT):
            seg_done = []
            for seg in plans[t]:
                semh = seg[-1]
                sem_count[id(semh)] = sem_count.get(id(semh), 0) + 16
                cs, ce = (-1, 0) if seg[0] == "wrap" else (seg[0], seg[1])
                seg_done.append((cs, ce, semh, sem_count[id(semh)]))
            for c in range(NCH):
                lo = c * CHUNK - 1
                hi = (c + 1) * CHUNK
                waits = {}
                for cs, ce, semh, cnt in seg_done:
                    if cs < hi and ce > lo:
                        key = id(semh)
                        if key not in waits or waits[key][1] < cnt:
                            waits[key] = (semh, cnt)
                g_waits[(t, c)] = list(waits.values())

        @block.sync
        def _(sync):
            def emit_loads(t, segs):
                rows = x[t * P : (t + 1) * P]
                s = t % 2
                for seg in segs:
                    semh = seg[-1]
                    if seg[0] == "wrap":
                        with nc.allow_non_contiguous_dma(reason="wrap col"):
                            sync.dma_start(
                                out=xt[s][:, 0:1], in_=rows[:, N - 1 : N]
                            ).then_inc(semh, 16)
                    else:
                        cs, ce = seg[0], seg[1]
                        sync.dma_start(
                            out=xt[s][:, 1 + cs : 1 + ce], in_=rows[:, cs:ce]
                        ).then_inc(semh, 16)

            emit_loads(0, plans[0][:2])       # wrap + first eighth
            sync.dma_start(out=vec[:], in_=vecd[:]).then_inc(LV, 16)
            sync.dma_start(out=ident[:], in_=identd[:]).then_inc(LV, 16)
            sync.dma_start(out=nident[:], in_=nidentd[:]).then_inc(LV, 16)
            emit_loads(0, plans[0][2:])
            emit_loads(1, plans[1])
            for idx in range(NIDX):
                t, c = divmod(idx, NCH)
                sync.wait_ge(V, idx + 1)
                sync.dma_start(
                    out=y[t * P : (t + 1) * P, c * CHUNK : (c + 1) * CHUNK],
                    in_=ym[idx % 4][:],
                ).then_inc(S[idx % 4], 16)
                # tile t+2 loads stream in as slot quarters free up:
                # store (t, 2q+2)'s V-wait implies y(t, 2q+2) done.
                if t + 2 < NT and c in (2, 4, 6, 7):
                    qi = {2: 0, 4: 1, 6: 2, 7: 3}[c]
                    segs = plans[t + 2]
                    if qi == 0:
                        emit_loads(t + 2, segs[:2])
                    else:
                        emit_loads(t + 2, segs[qi + 1 : qi + 2])

        bb_after_pe = {}

        @block.tensor
        def _(tensor):
            # broadcast pairs: p = 2*rr + (0: -mean, 1: ks), rr a 1024-range
            tensor.wait_ge(LV, 48)
            bb = 0
            for p in range(2 * NCH):
                rr, which = divmod(p, 2)
                src = msp if which == 0 else ksp
                if p >= 2:
                    tensor.wait_ge(C, p - 1)  # ACT copied ps[p%2], reusable
                for h in range(2):
                    r0 = rr * CHUNK + h * R
                    tensor.matmul(
                        ps[p % 2][:, h * R : (h + 1) * R],
                        ones,
                        src[:, r0 : r0 + R],
                        start=True,
                        stop=True,
                    ).then_inc(BB, 1)
                    bb += 1
            # d on PE straight from xt: psd = I@x[c0+1:] + (-I)@x[c0:]
            # + ones3@msp (the -mean splits).  Exact: identity matmuls touch
            # one operand element per output, so PSUM rounding matches the
            # two-op tensor_tensor path.
            for i in pe_list:
                t, c = divmod(i, NCH)
                c0 = c * CHUNK
                for semh, thresh in g_waits[(t, c)]:
                    tensor.wait_ge(semh, thresh)
                r = pe_rank[i]
                if r >= 2:
                    tensor.wait_ge(A, pe_list[r - 2] + 1)  # psd[r%2] consumed
                for h in range(2):
                    hs = slice(h * R, (h + 1) * R)
                    x1 = xt[t % 2][:, c0 + 1 + h * R : c0 + 1 + (h + 1) * R]
                    x0 = xt[t % 2][:, c0 + h * R : c0 + (h + 1) * R]
                    tensor.matmul(
                        psd[r % 2][:, hs], ident[:], x1, start=True, stop=False
                    ).then_inc(BB, 1)
                    tensor.matmul(
                        psd[r % 2][:, hs], nident[:], x0, start=False, stop=False
                    ).then_inc(BB, 1)
                    tensor.matmul(
                        psd[r % 2][:, hs], ones,
                        msp[:, c0 + h * R : c0 + (h + 1) * R],
                        start=False, stop=True,
                    ).then_inc(BB, 1)
                    bb += 3
                bb_after_pe[i] = bb

        @block.scalar
        def _(scalar):
            q = 0

            def copies(k):
                nonlocal q
                for _ in range(k):
                    rr, which = divmod(q, 2)
                    dst = mean_b if which == 0 else ks_b
                    scalar.wait_ge(BB, 2 * q + 2)  # both halves of pair q
                    scalar.activation(
                        dst[:, rr * CHUNK : (rr + 1) * CHUNK], ps[q % 2][:], Copy
                    ).then_inc(C, 1)
                    q += 1

            for idx in range(NIDX):
                if q < 2 * NCH:
                    copies(2)
                if idx in pe_set:
                    scalar.wait_ge(BB, bb_after_pe[idx])
                    if idx >= 2:
                        scalar.wait_ge(Mm, idx - 1)  # db[idx%2] free
                    a_src = psd[pe_rank[idx] % 2][:]
                else:
                    if idx in pd_set:
                        scalar.wait_ge(DP, ndp(idx))
                    else:
                        scalar.wait_ge(DD, ndd(idx))
                    # in place: d producers already synced on db[idx%2]
                    a_src = db[idx % 2][:]
                scalar.activation(db[idx % 2][:], a_src, Abs).then_inc(A, 1)

        @block.gpsimd
        def _(gpsimd):
            # step i: g(i), then d(i-1) when (i-1) is a Pool-d chunk
            for i in range(NIDX + 1):
                if i < NIDX and i not in dg_set and i not in pe_set:
                    t, c = divmod(i, NCH)
                    for semh, thresh in g_waits[(t, c)]:
                        gpsimd.wait_ge(semh, thresh)
                    gb_release_wait(gpsimd, i)
                    if drains:
                        gpsimd.drain()  # WAR vs own d reads of gb
                    c0 = c * CHUNK
                    gpsimd.tensor_tensor(
                        gb[i % 3][:],
                        xt[t % 2][:, c0 + 1 : c0 + CHUNK + 1],
                        xt[t % 2][:, c0 : c0 + CHUNK],
                        sub,
                    ).then_inc(PG, 1)
                j = i - 1
                if 0 <= j < NIDX and j in pd_set:
                    tj, cj = divmod(j, NCH)
                    cj0 = cj * CHUNK
                    gpsimd.wait_ge(C, 2 * cj + 1)
                    if j >= 2:
                        gpsimd.wait_ge(Mm, j - 1)  # db[j%2] free (m(j-2) done)
                    if drains:
                        gpsimd.drain()  # RAW: reads gb[j%3] from own g(j)
                    # mean_b holds -mean, so d = g + mean_b
                    gpsimd.tensor_tensor(
                        db[j % 2][:], gb[j % 3][:],
                        mean_b[:, cj0 : cj0 + CHUNK], add,
                    ).then_inc(DP, 1)

        @block.vector
        def _(vector):
            # step i: m(i-2), d(i), y(i-3)   (m before d: db[i%2] WAR)
            for i in range(NIDX + 3):
                j = i - 2
                if 0 <= j < NIDX:
                    tj, cj = divmod(j, NCH)
                    vector.wait_ge(A, j + 1)
                    vector.wait_ge(C, 2 * cj + 2)
                    if drains:
                        vector.drain()  # mb[j%2] WAR vs y(j-2); db read
                    vector.tensor_tensor(
                        mb[j % 2][:],
                        db[j % 2][:],
                        ks_b[:, cj * CHUNK : (cj + 1) * CHUNK],
                        is_le,
                    ).then_inc(Mm, 1)
                if i < NIDX and i in dg_set:
                    t, c = divmod(i, NCH)
                    c0 = c * CHUNK
                    for semh, thresh in g_waits[(t, c)]:
                        vector.wait_ge(semh, thresh)
                    if drains:
                        vector.drain()
                    vector.tensor_tensor(
                        gb[i % 3][:],
                        xt[t % 2][:, c0 + 1 : c0 + CHUNK + 1],
                        xt[t % 2][:, c0 : c0 + CHUNK],
                        sub,
                    )
                if i < NIDX and i not in pd_set and i not in pe_set:
                    t, c = divmod(i, NCH)
                    c0 = c * CHUNK
                    if i not in dg_set:
                        vector.wait_ge(PG, pg_count(i))
                    vector.wait_ge(C, 2 * c + 1)
                    # gb[i%3] anti-dep vs the d 3 steps back is already
                    # ordered: that d ran on DVE/Pool before this step's g.
                    # db[i%2] free: m(i-2) precedes on this engine.
                    if drains:
                        vector.drain()
                    vector.tensor_tensor(
                        db[i % 2][:], gb[i % 3][:], mean_b[:, c0 : c0 + CHUNK], add
                    ).then_inc(DD, 1)
                jy = i - 3
                if 0 <= jy < NIDX:
                    ty, cy = divmod(jy, NCH)
                    cy0 = cy * CHUNK
                    vector.wait_ge(Mm, jy + 1)
                    if jy >= 4:
                        vector.wait_ge(S[jy % 4], 16 * (jy // 4))  # ym free
                    if drains:
                        vector.drain()
                    vector.tensor_tensor(
                        ym[jy % 4][:],
                        mb[jy % 2][:],
                        xt[ty % 2][:, cy0 + 1 : cy0 + CHUNK + 1],
                        mult,
                    ).then_inc(V, 1)

    return nc


def _host_vectors(mean_grad, var_grad, k):
    import ml_dtypes

    mg = np.asarray(mean_grad, dtype=np.float32)
    vg = np.asarray(var_grad, dtype=np.float32)
    kf = np.float32(k)
    ks = (kf * np.sqrt(vg, dtype=np.float32)).astype(np.float32)

    def split3(v):
        hi = v.astype(ml_dtypes.bfloat16)
        r1 = v - hi.astype(np.float32)
        mid = r1.astype(ml_dtypes.bfloat16)
        r2 = r1 - mid.astype(np.float32)
        lo = r2.astype(ml_dtypes.bfloat16)
        return np.stack([hi, mid, lo])

    vec = np.empty((3, 2 * N + P), dtype=ml_dtypes.bfloat16)
    vec[:, 0:N] = split3(-mg)
    vec[:, N : 2 * N] = split3(ks)
    vec[:, 2 * N :] = np.ones((3, P), dtype=ml_dtypes.bfloat16)
    return vec


_IDENT = np.eye(P, dtype=np.float32)
_NIDENT = -np.eye(P, dtype=np.float32)


class _FastRunner:
    """Cached PJRT dispatch (axon path).

    run_bass_kernel_spmd -> run_bass_via_pjrt rebuilds jax.jit(shard_map(...))
    every call (retrace), transfers 128 MiB of host zeros for the donated
    outputs, and splits/reconcatenates the output.  This does the lowering
    once, keeps the compiled callable, creates the donated zeros on device,
    and feeds the full [4096, 8192] input directly.
    """

    def __init__(self, nc, n_cores):
        import jax
        import jax.numpy as jnp
        from jax.sharding import Mesh, NamedSharding, PartitionSpec
        from jax.experimental.shard_map import shard_map
        from concourse import bass2jax
        import concourse.mybir as mybir

        bass2jax.install_neuronx_cc_hook()
        in_names = []
        out_names = []
        out_avals = []
        zero_shapes = []
        partition_name = (
            nc.partition_id_tensor.name if nc.partition_id_tensor else None
        )
        for alloc in nc.m.functions[0].allocations:
            if not isinstance(alloc, mybir.MemoryLocationSet):
                continue
            name = alloc.memorylocations[0].name
            if alloc.kind == "ExternalInput":
                if name != partition_name:
                    in_names.append(name)
            elif alloc.kind == "ExternalOutput":
                shape = tuple(alloc.tensor_shape)
                dtype = mybir.dt.np(alloc.dtype)
                out_names.append(name)
                out_avals.append(jax.core.ShapedArray(shape, dtype))
                zero_shapes.append((shape, dtype))
        if nc.dbg_addr is not None:
            raise RuntimeError("debug nc unsupported in fast path")
        self.in_names = in_names
        n_params = len(in_names)
        n_outs = len(out_names)
        all_in_names = list(in_names) + list(out_names)
        if partition_name is not None:
            all_in_names.append(partition_name)

        def _body(*args):
            operands = list(args)
            if partition_name is not None:
                operands.append(bass2jax.partition_id_tensor())
            outs = bass2jax._bass_exec_p.bind(
                *operands,
                out_avals=tuple(out_avals),
                in_names=tuple(all_in_names),
                out_names=tuple(out_names),
                lowering_input_output_aliases=(),
                sim_require_finite=True,
                sim_require_nnan=True,
                nc=nc,
            )
            return tuple(outs)

        devices = jax.devices()[:n_cores]
        assert len(devices) == n_cores, len(jax.devices())
        mesh = Mesh(np.asarray(devices), ("core",))
        spec = PartitionSpec("core")
        self._sharded = jax.jit(
            shard_map(
                _body,
                mesh=mesh,
                in_specs=(spec,) * (n_params + n_outs),
                out_specs=(spec,) * n_outs,
                check_rep=False,
            ),
            donate_argnums=tuple(range(n_params, n_params + n_outs)),
            keep_unused=True,
        )
        sharding = NamedSharding(mesh, spec)
        self._make_zeros = jax.jit(
            lambda: tuple(
                jnp.zeros((n_cores * s[0], *s[1:]), d) for s, d in zero_shapes
            ),
            out_shardings=(sharding,) * n_outs,
        )

    def __call__(self, *global_inputs):
        zeros = self._make_zeros()
        outs = self._sharded(*global_inputs, *zeros)
        return [np.asarray(o) for o in outs]


_CACHE = {}


def _run_fallback(nc, x, vec):
    from concourse.bass_utils import run_bass_kernel_spmd

    in_maps = [
        {
            "x": x[i * ROWS : (i + 1) * ROWS],
            "vecd": vec,
            "identd": _IDENT,
            "nidentd": _NIDENT,
        }
        for i in range(N_CORES)
    ]
    res = run_bass_kernel_spmd(nc, in_maps, core_ids=list(range(N_CORES)))
    return np.concatenate([res.results[i]["y"] for i in range(N_CORES)], axis=0)


def kernel(output, mean_grad, var_grad, k):
    x = np.ascontiguousarray(np.asarray(output, dtype=np.float32))
    assert x.shape == (B, N), x.shape
    vec = _host_vectors(mean_grad, var_grad, k)

    if "nc" not in _CACHE:
        _CACHE["nc"] = build_nc(drains=False)
    nc = _CACHE["nc"]

    try:
        if "runner" not in _CACHE:
            _CACHE["runner"] = _FastRunner(nc, N_CORES)
        runner = _CACHE["runner"]
        vec8 = np.ascontiguousarray(np.tile(vec, (N_CORES, 1)))
        ins = {
            "x": x,
            "vecd": vec8,
            "identd": np.tile(_IDENT, (N_CORES, 1)),
            "nidentd": np.tile(_NIDENT, (N_CORES, 1)),
        }
        outs = runner(*[ins[nm] for nm in runner.in_names])
        return outs[0]
    except Exception:
        _CACHE.pop("runner", None)
        return _run_fallback(nc, x, vec)
